# revision 23
# baseline (speedup 1.0000x reference)
"""MoE (16 routed experts, top-2, + shared expert) on 8 Trainium2 cores — v3.

Expert-parallel with host-side routing (gate + gather/scatter on host):
  - Core c owns 2 routed experts (host assigns expert pairs to balance the
    token load); tokens for slot A occupy x columns [0, CA), slot B
    [CA, CA+CB), where CA/CB are the max padded counts across cores.
  - Routed experts run in scaled e4m3 fp8 with DoubleRow matmuls
    (~2x PE rate, half the weight bytes).  Their outputs are multiplied
    by the top-2 softmax weights (~0.1-0.3), so fp8 noise is attenuated
    ~5x in the final output; measured rel err ~1.6e-2 vs the 2e-2 gate.
  - The shared expert (full-scale contribution) stays in f16 and is
    hybrid-sharded: cores pair up; each pair owns a 512-token block and
    each core of the pair computes one half of HS (halving the f16
    weight DMA vs full replication); the host sums the two partials.
  - All outputs return as f16; host applies combine weights and
    scatter-adds in f32.
  - DMAs are consolidated into 2 input + 2 output transfers per
    iteration (one e4m3 input blob, one f16 input blob) because every
    dma_start serializes ~0.6us on the shared HWDGE dispatcher.

Layouts are transposed (tokens along the matmul free dim):
    zT = W2^T @ (u * silu(g)),  [u;g]^T = W1^T @ xT
so no on-chip transposes are needed.  Weights arrive pre-packed per-core
from the host in the exact SBUF tile layout (contiguous per-partition runs).
"""

import sys

for _p in ("/opt/trn_rl_repo", "/root/.axon_site/_ro/trn_rl_repo"):
    if _p not in sys.path:
        sys.path.insert(0, _p)

import contextlib
import os

import numpy as np
import ml_dtypes

import concourse.bass as bass  # noqa: F401
import concourse.tile as tile
from concourse import bacc, mybir
from concourse.bass_utils import run_bass_kernel_spmd

try:
    from antenv import axon_hooks as _axon_hooks  # noqa: F401
except ImportError:
    os.environ.setdefault("BASS_NEVER_TRACE", "1")

B, S, D = 2, 1024, 1024
H = 512            # routed expert hidden
HS = 1024          # shared expert hidden
E = 16
ROUTE_SCALE = 1.0
T = B * S
N_CORES = 8
EPC = E // N_CORES
TDS = 512                   # shared-expert tokens per core (pair-block)
P = 128
KD = D // P                 # fc1 contraction chunks (8)
MD = D // P                 # fc2 output chunks (8)
NH = H // P                 # routed hidden chunks (4)
NHS2 = HS // 2 // P         # shared hidden chunks per core (HS half) (4)
PAD = 16                    # token-count granularity (DoubleRow step%16)
CBLK = 512                  # max psum free dim (one 2KB bank of f32)

SW1 = 16.0                  # fp8 pre-scales (keep |h| < 240 for ieee e4m3)
SXR = 1.0
SW2 = 16.0

F32 = mybir.dt.float32
F16 = mybir.dt.float16
E4 = mybir.dt.float8e4
NP_E4 = ml_dtypes.float8_e4m3
ACT = mybir.ActivationFunctionType

W1N = NH * 2 * (KD // 2) * 2 * P      # 8192 e4m3 elems / partition
W2N = MD * (NH // 2) * 2 * P          # 4096
XSN = KD * TDS                        # 4096 f16 elems / partition
WS1N = NHS2 * 2 * KD * P              # 8192 f16
WS2N = MD * NHS2 * P                  # 4096 f16

LAST_RESULTS = None
_NC_CACHE = {}


def _blocks(cap):
    """Split cap columns into blocks of <= CBLK, sizes multiple of PAD."""
    nb = -(-cap // CBLK)
    base = -(-cap // (nb * PAD)) * PAD
    out = []
    off = 0
    while off < cap:
        n = min(base, cap - off)
        out.append((off, n))
        off += n
    return out


def _build_nc(CA, CB, reps=1, static_loop=False, only=None, qconf="a"):
    # only='dma' -> input DMAs + dummy output DMAs only
    # qconf 'a': SP=[inA,inB] Act=[outA,outB]
    # qconf 'b': SP=[inA] Pool=[inB] Act=[outA,outB]
    nc = bacc.Bacc(None, target_bir_lowering=False)
    C2 = CA + CB

    XRN = KD * C2
    NA = XRN + 2 * (W1N + W2N)
    NB = XSN + WS1N + WS2N
    ina = nc.declare_dram_parameter("inA", [P, NA], E4, isOutput=False)
    inb = nc.declare_dram_parameter("inB", [P, NB], F16, isOutput=False)
    outa = nc.declare_dram_parameter("outA", [P, MD * C2], E4, isOutput=True)
    outb = nc.declare_dram_parameter("outB", [P, MD * TDS], F16, isOutput=True)

    slot_cols = [(0, CA), (CA, CB)]

    def v_w1(ap):
        return ap.rearrange("p (h s k j m) -> p h s k j m", h=NH, s=2, k=KD // 2, j=2)

    def v_w2(ap):
        return ap.rearrange("p (d k j m) -> p d k j m", d=MD, k=NH // 2, j=2)

    def v_ws1(ap):
        return ap.rearrange("p (h s k m) -> p h s k m", h=NHS2, s=2, k=KD)

    def v_ws2(ap):
        return ap.rearrange("p (d k m) -> p d k m", d=MD, k=NHS2)

    with tile.TileContext(nc) as tc:
        with (
            tc.tile_pool(name="apool", bufs=2) as apool,
            tc.tile_pool(name="bpool", bufs=2) as bpool,
            tc.tile_pool(name="hpool", bufs=2) as hpool,
            tc.tile_pool(name="spool", bufs=4) as spool,
            tc.tile_pool(name="opool", bufs=2) as opool,
            tc.tile_pool(name="pp", bufs=2, space="PSUM") as pp,
        ):
            pre = {}
            if only == "nodma":
                # hoist input DMAs out of the loop: isolates compute speed
                # from concurrent input DMA traffic
                t = apool.tile([P, NA], E4, tag="inA", name="pre_inA", bufs=1)
                nc.sync.dma_start(t[:], ina.ap())
                pre["ta"] = t
                t = bpool.tile([P, NB], F16, tag="inB", name="pre_inB", bufs=1)
                nc.sync.dma_start(t[:], inb.ap())
                pre["tb"] = t
            if reps > 1 and not static_loop:
                # unroll inside the hw loop: consecutive emissions alternate
                # pool buffers (bufs=2), enabling cross-emission DMA/compute
                # overlap.  For_i has an all-engine barrier at the backedge,
                # so deeper unroll amortizes the pipeline fill/drain cost.
                n_emit = next(u for u in (16, 8, 4, 2, 1) if reps % u == 0)
                loop_cm = tc.For_i(0, reps // n_emit, 1)
            else:
                n_emit = reps if (static_loop and reps > 1) else 1
                loop_cm = contextlib.nullcontext()
            with loop_cm:
              for _rep in range(n_emit):
                if only == "nodma":
                    ta, tb = pre["ta"], pre["tb"]
                else:
                    ta = apool.tile([P, NA], E4, tag="inA", name="inA_t")
                    nc.sync.dma_start(ta[:], ina.ap())
                    tb = bpool.tile([P, NB], F16, tag="inB", name="inB_t")
                    inb_q = nc.sync if qconf == "a" else nc.gpsimd
                    inb_q.dma_start(tb[:], inb.ap())

                xr_t = ta[:, 0:XRN].rearrange("p (k c) -> p k c", k=KD)
                off = XRN
                w1ts, w2ts = [], []
                for slot in range(EPC):
                    w1ts.append(v_w1(ta[:, off:off + W1N]))
                    off += W1N
                    w2ts.append(v_w2(ta[:, off:off + W2N]))
                    off += W2N

                xs_t = tb[:, 0:XSN].rearrange("p (k c) -> p k c", k=KD)
                ws1t = v_ws1(tb[:, XSN:XSN + WS1N])
                ws2t = v_ws2(tb[:, XSN + WS1N:XSN + WS1N + WS2N])

                if only == "dma":
                    oa = opool.tile([P, MD * C2], E4, tag="oa", name="oa_dma")
                    nc.vector.memset(oa[:], 0.0)
                    nc.scalar.dma_start(outa.ap(), oa[:])
                    ob = opool.tile([P, MD * TDS], F16, tag="ob", name="ob_dma")
                    nc.vector.memset(ob[:], 0.0)
                    nc.scalar.dma_start(outb.ap(), ob[:])
                    continue

                oa_flat = opool.tile([P, MD * C2], E4, tag="oa", name="oa")
                ob_flat = opool.tile([P, MD * TDS], F16, tag="ob", name="ob")
                ob_t = ob_flat[:].rearrange("p (d c) -> p d c", d=MD)
                h_ts = [hpool.tile([P, NH, slot_cols[s][1]], E4, tag=f"hr{s}",
                                   name=f"h{s}") for s in range(EPC)]
                hs_t = hpool.tile([P, NHS2, TDS], F16, tag="hs", name="hs")
                o_ts = [
                    oa_flat[:, MD * slot_cols[s][0]:
                            MD * (slot_cols[s][0] + slot_cols[s][1])].rearrange(
                        "p (d c) -> p d c", d=MD) for s in range(EPC)]

                # group emitters: interleaved so PE alternates between short
                # routed groups (drain-bound) and long shared groups
                # (PE-bound), keeping the tensor engine continuously busy
                # (its clock ramps down after any idle gap).
                def r_f1(slot, hc):
                    cap = slot_cols[slot][1]
                    ps_u = pp.tile([P, cap], F32, tag="pru", name="ps_u")
                    ps_g = pp.tile([P, cap], F32, tag="prg", name="ps_g")
                    for ps, half in ((ps_u, 0), (ps_g, 1)):
                        for kp in range(KD // 2):
                            nc.tensor.matmul(
                                ps[:], w1ts[slot][:, hc, half, kp],
                                xr_t[:, 2 * kp:2 * kp + 2,
                                     slot_cols[slot][0]:slot_cols[slot][0] + cap],
                                start=(kp == 0), stop=(kp == KD // 2 - 1),
                                perf_mode=mybir.MatmulPerfMode.DoubleRow)
                    sil = spool.tile([P, cap], F32, tag="sil", name="sil")
                    nc.scalar.activation(sil[:], ps_g[:], ACT.Silu,
                                         scale=1.0 / (SW1 * SXR))
                    nc.vector.tensor_mul(h_ts[slot][:, hc], ps_u[:], sil[:])

                def s_f1(hc):
                    ps_u = pp.tile([P, TDS], F32, tag="psu", name="ps_us")
                    ps_g = pp.tile([P, TDS], F32, tag="psg", name="ps_gs")
                    for ps, half in ((ps_u, 0), (ps_g, 1)):
                        for k in range(KD):
                            nc.tensor.matmul(ps[:], ws1t[:, hc, half, k],
                                             xs_t[:, k],
                                             start=(k == 0), stop=(k == KD - 1))
                    sil = spool.tile([P, TDS], F32, tag="sil", name="sil")
                    nc.scalar.activation(sil[:], ps_g[:], ACT.Silu)
                    nc.vector.tensor_mul(hs_t[:, hc], ps_u[:], sil[:])

                def r_f2(slot, dp):
                    cap = slot_cols[slot][1]
                    ps_z = pp.tile([P, cap], F32,
                                   tag=("pru", "prg")[dp % 2], name="ps_z")
                    for kp in range(NH // 2):
                        nc.tensor.matmul(
                            ps_z[:], w2ts[slot][:, dp, kp],
                            h_ts[slot][:, 2 * kp:2 * kp + 2],
                            start=(kp == 0), stop=(kp == NH // 2 - 1),
                            perf_mode=mybir.MatmulPerfMode.DoubleRow)
                    if dp % 2 == 0:
                        nc.scalar.activation(o_ts[slot][:, dp], ps_z[:],
                                             ACT.Copy,
                                             scale=1.0 / (SW1 * SXR * SW2))
                    else:
                        nc.vector.tensor_scalar_mul(
                            o_ts[slot][:, dp], ps_z[:],
                            1.0 / (SW1 * SXR * SW2))

                def s_f2(dp):
                    ps_z = pp.tile([P, TDS], F32,
                                   tag=("psu", "psg")[dp % 2], name="ps_zs")
                    for k in range(NHS2):
                        nc.tensor.matmul(ps_z[:], ws2t[:, dp, k],
                                         hs_t[:, k],
                                         start=(k == 0), stop=(k == NHS2 - 1))
                    if dp % 2 == 0:
                        nc.scalar.activation(ob_t[:, dp], ps_z[:], ACT.Copy)
                    else:
                        nc.vector.tensor_copy(ob_t[:, dp], ps_z[:])

                do_r = only not in ("norouted",)
                do_s = only not in ("noshared",)
                if do_r and do_s:
                    # phase 1: fc1 groups, shared drains trail into phase 2
                    r_f1(0, 0); r_f1(0, 1); s_f1(0)
                    r_f1(0, 2); r_f1(0, 3); s_f1(1)
                    r_f1(1, 0); r_f1(1, 1); s_f1(2)
                    s_f1(3); r_f1(1, 2); r_f1(1, 3)
                    # phase 2: fc2 groups
                    r_f2(0, 0); r_f2(0, 1); s_f2(0)
                    r_f2(0, 2); r_f2(0, 3); s_f2(1)
                    r_f2(0, 4); r_f2(0, 5); s_f2(2)
                    r_f2(0, 6); r_f2(0, 7); s_f2(3)
                    r_f2(1, 0); r_f2(1, 1); s_f2(4)
                    r_f2(1, 2); r_f2(1, 3); s_f2(5)
                    r_f2(1, 4); r_f2(1, 5); s_f2(6)
                    r_f2(1, 6); r_f2(1, 7)
                    nc.scalar.dma_start(outa.ap(), oa_flat[:])
                    s_f2(7)
                    nc.scalar.dma_start(outb.ap(), ob_flat[:])
                elif do_r:
                    for s in range(EPC):
                        for hc in range(NH):
                            r_f1(s, hc)
                    for s in range(EPC):
                        for dp in range(MD):
                            r_f2(s, dp)
                    nc.scalar.dma_start(outa.ap(), oa_flat[:])
                    nc.vector.memset(ob_flat[:], 0.0)
                    nc.scalar.dma_start(outb.ap(), ob_flat[:])
                else:
                    for hc in range(NHS2):
                        s_f1(hc)
                    for dp in range(MD):
                        s_f2(dp)
                    nc.scalar.dma_start(outb.ap(), ob_flat[:])
                    nc.vector.memset(oa_flat[:], 0.0)
                    nc.scalar.dma_start(outa.ap(), oa_flat[:])
    nc.finalize()
    return nc


def _route(xf, Wg):
    logits = xf @ Wg.T
    m = logits.max(axis=-1, keepdims=True)
    p = np.exp(logits - m)
    scores = p / p.sum(axis=-1, keepdims=True)
    i1 = scores.argmax(axis=-1)
    rows = np.arange(T)
    s1 = scores[rows, i1]
    masked = scores.copy()
    masked[rows, i1] = -np.inf
    i2 = masked.argmax(axis=-1)
    s2 = scores[rows, i2]
    return i1, s1 * ROUTE_SCALE, i2, s2 * ROUTE_SCALE


def _pack_w1_r(W1e):
    """[D, 2H] -> [P, NH, 2, KD/2, 2, P] e4m3 scaled (pair-interleaved k)."""
    A = (W1e * SW1).reshape(KD // 2, 2, P, 2, NH, P)   # [kp, j, ki, half, hc, m]
    return np.ascontiguousarray(
        A.transpose(2, 4, 3, 0, 1, 5).reshape(P, -1)
    ).astype(NP_E4)


def _pack_w2_r(W2e):
    """[H, D] -> [P, MD, NH/2, 2, P] e4m3 scaled."""
    A = (W2e * SW2).reshape(NH // 2, 2, P, MD, P)      # [kp, j, ki, dp, m]
    return np.ascontiguousarray(
        A.transpose(2, 3, 0, 1, 4).reshape(P, -1)
    ).astype(NP_E4)


def _pack_ws1_half(Ws1, h):
    """[D, 2HS] half h -> [P, NHS2, 2, KD, P] f16."""
    u = Ws1[:, h * (HS // 2):(h + 1) * (HS // 2)]
    g = Ws1[:, HS + h * (HS // 2):HS + (h + 1) * (HS // 2)]
    A = np.stack([u, g], axis=0)                       # [half, D, HS/2]
    A = A.reshape(2, KD, P, NHS2, P)                   # [half, ko, ki, hc, m]
    return np.ascontiguousarray(
        A.transpose(2, 3, 0, 1, 4).reshape(P, -1)).astype(np.float16)


def _pack_ws2_half(Ws2, h):
    """[HS, D] rows half h -> [P, MD, NHS2, P] f16."""
    A = Ws2[h * (HS // 2):(h + 1) * (HS // 2), :]      # [HS/2, D]
    A = A.reshape(NHS2, P, MD, P)                      # [ko, ki, dp, m]
    return np.ascontiguousarray(
        A.transpose(1, 2, 0, 3).reshape(P, -1)).astype(np.float16)


def _pack_x(cols_f32, C, dtype, scale):
    """[D, n] -> [P, KD, C] (zero-padded to C columns)."""
    n = cols_f32.shape[1]
    out = np.zeros((P, KD, C), dtype=dtype)
    v = (cols_f32 * scale).reshape(KD, P, n).transpose(1, 0, 2)
    out[:, :, :n] = v.astype(dtype)
    return out


def prepare(x, Wg, W1, W2, Ws1, Ws2):
    """Host routing, balancing, packing. Returns (in_maps, meta)."""
    x = np.asarray(x, dtype=np.float32)
    Wg = np.asarray(Wg, dtype=np.float32)
    W1 = np.asarray(W1, dtype=np.float32)
    W2 = np.asarray(W2, dtype=np.float32)
    Ws1 = np.asarray(Ws1, dtype=np.float32)
    Ws2 = np.asarray(Ws2, dtype=np.float32)

    xf = np.ascontiguousarray(x.reshape(T, D))
    i1, s1, i2, s2 = _route(xf, Wg)

    toks, wts = [], []
    for e in range(E):
        sel = np.where((i1 == e) | (i2 == e))[0]
        toks.append(sel)
        wts.append(np.where(i1[sel] == e, s1[sel], s2[sel]).astype(np.float32))

    counts = np.array([len(t) for t in toks])
    order = np.argsort(-counts)                     # big..small
    slotA = [int(order[c]) for c in range(N_CORES)]           # biggest 8
    slotB = [int(order[2 * N_CORES - 1 - c]) for c in range(N_CORES)]
    npad = [-(-c // PAD) * PAD for c in counts]
    CA = max(PAD, max(npad[e] for e in slotA))
    CB = max(PAD, max(npad[e] for e in slotB))

    w1p = [_pack_w1_r(W1[e]) for e in range(E)]
    w2p = [_pack_w2_r(W2[e]) for e in range(E)]
    ws1h = [_pack_ws1_half(Ws1, h) for h in range(2)]
    ws2h = [_pack_ws2_half(Ws2, h) for h in range(2)]
    in_maps = []
    for c in range(N_CORES):
        eA, eB = slotA[c], slotB[c]
        xcat = np.zeros((P, KD, CA + CB), dtype=NP_E4)
        xcat[:, :, :CA] = _pack_x(xf[toks[eA]].T, CA, NP_E4, SXR)
        xcat[:, :, CA:] = _pack_x(xf[toks[eB]].T, CB, NP_E4, SXR)
        ina = np.concatenate(
            [xcat.reshape(P, -1), w1p[eA], w2p[eA], w1p[eB], w2p[eB]], axis=1)
        blk, hh = c // 2, c % 2
        xsp = _pack_x(xf[blk * TDS:(blk + 1) * TDS].T, TDS, np.float16,
                      1.0).reshape(P, -1)
        inb = np.concatenate([xsp, ws1h[hh], ws2h[hh]], axis=1)
        in_maps.append({"inA": ina, "inB": inb})
    meta = dict(CA=CA, CB=CB, slotA=slotA, slotB=slotB, toks=toks, wts=wts)
    return in_maps, meta


def kernel(x, Wg, W1, W2, Ws1, Ws2):
    global LAST_RESULTS
    in_maps, meta = prepare(x, Wg, W1, W2, Ws1, Ws2)
    CA, CB = meta["CA"], meta["CB"]

    key = (CA, CB)
    if key not in _NC_CACHE:
        _NC_CACHE[key] = _build_nc(CA, CB)
    nc = _NC_CACHE[key]

    try:
        LAST_RESULTS = run_bass_kernel_spmd(nc, in_maps, list(range(N_CORES)))
    except Exception:
        LAST_RESULTS = run_bass_kernel_spmd(nc, in_maps, list(range(N_CORES)))
    res = LAST_RESULTS.results

    toks, wts = meta["toks"], meta["wts"]
    out = np.zeros((T, D), dtype=np.float32)
    for c in range(N_CORES):
        oa = np.asarray(res[c]["outA"], dtype=np.float32)
        for slot, e in ((0, meta["slotA"][c]), (1, meta["slotB"][c])):
            n = len(toks[e])
            coff, cap = ((0, CA), (CA, CB))[slot]
            zrf = oa[:, MD * coff:MD * (coff + cap)].reshape(P, MD, cap)
            zt = zrf[:, :, :n].transpose(1, 0, 2).reshape(D, n)       # [D, n]
            out[toks[e]] += wts[e][:, None] * zt.T
        blk = c // 2
        zsf = np.asarray(res[c]["outB"], dtype=np.float32).reshape(P, MD, TDS)
        out[blk * TDS:(blk + 1) * TDS] += zsf.transpose(1, 0, 2).reshape(D, TDS).T
    return out.reshape(B, S, D)
